# revision 1
# baseline (speedup 1.0000x reference)
"""Trainium2 Bass kernel for nn_DGM_c (DGM graph-construction layer).

Reference computation (see problem statement):
    x_emb = relu(A @ x @ W + b)                       [B,N,E]
    centroid = mean(x_emb, axis=-2); scale = 0.9/max|x_emb-centroid|
    xs = (x_emb-centroid)*scale
    D = cdist(xs)^2 ; adj = sigmoid(T*(|thr| - D))
    edge_index = fixed arange pattern ; edges_weight = adj.reshape(-1)

Key identity: the centroid cancels in pairwise differences, so
    D_ij = scale^2 * (|e_i|^2 + |e_j|^2 - 2 e_i.e_j),  e = x_emb.
Only one global scalar (scale) couples all rows, so we run two SPMD
launches over 8 NeuronCores (core c -> batch c//4, row-block c%4 of 512
rows) with a tiny host step between them:

  Launch 1 (per core): t^T = x_b^T A_blk^T via 16 accumulated fp32
    matmuls (A pre-transposed on host so the contraction dim lands on
    partitions), then x_emb_blk^T = relu(W^T t^T + b) -> [64, 512].
  Host: assemble x_emb, compute scale/sq norms, fold every constant into
    small augmented operands.
  Launch 2 (per core): one K=65 matmul per [128,512] tile computes
    2*T*scale^2*G - T*scale^2*sq_j; ScalarE applies
    sigmoid(psum + (T|thr| - T*scale^2*sq_i)) and the result is DMAed out.

edge_index is input-independent -> generated host-side.
"""

import os
import sys
from contextlib import ExitStack

for _p in ("/opt/trn_rl_repo", "/root/.axon_site/_ro/trn_rl_repo"):
    if os.path.isdir(_p) and _p not in sys.path:
        sys.path.insert(0, _p)

import numpy as np

import concourse.bass as bass  # noqa: F401  (registers engines)
import concourse.tile as tile
from concourse import bacc, mybir
from concourse.bass_utils import run_bass_kernel_spmd

B, N, F_IN, F_EMB = 2, 2048, 128, 64
NCORES = 8
CPB = NCORES // B          # cores per batch
R = N // CPB               # rows per core = 512
KT = N // 128              # contraction tiles = 16
IT = R // 128              # row tiles per core = 4
JT = N // 512              # column chunks per core = 4
F32 = mybir.dt.float32
CORE_IDS = list(range(NCORES))

_NC_CACHE: dict = {}


def _build_phase1():
    """x_emb_blk^T = relu(W^T (x_b^T A_blk^T) + b) for this core's rows."""
    nc = bacc.Bacc("TRN2", target_bir_lowering=False, debug=False,
                   num_devices=NCORES)
    at_ap = nc.dram_tensor("at", [N, R], F32, kind="ExternalInput").ap()
    x_ap = nc.dram_tensor("x", [N, F_IN], F32, kind="ExternalInput").ap()
    w_ap = nc.dram_tensor("w", [F_IN, F_EMB], F32, kind="ExternalInput").ap()
    bb_ap = nc.dram_tensor("bb", [F_EMB, 1], F32, kind="ExternalInput").ap()
    et_ap = nc.dram_tensor("et", [F_EMB, R], F32, kind="ExternalOutput").ap()

    with tile.TileContext(nc) as tc, ExitStack() as ctx:
        const = ctx.enter_context(tc.tile_pool(name="const", bufs=1))
        xpool = ctx.enter_context(tc.tile_pool(name="xp", bufs=1))
        apool = ctx.enter_context(tc.tile_pool(name="ap", bufs=4))
        pst = ctx.enter_context(tc.tile_pool(name="pst", bufs=1, space="PSUM"))
        pse = ctx.enter_context(tc.tile_pool(name="pse", bufs=1, space="PSUM"))
        spool = ctx.enter_context(tc.tile_pool(name="sp", bufs=2))

        wsb = const.tile([F_IN, F_EMB], F32)
        nc.sync.dma_start(wsb[:], w_ap[:])
        bsb = const.tile([F_EMB, 1], F32)
        nc.sync.dma_start(bsb[:], bb_ap[:])

        # x_b as 16 lhsT tiles: partition = node-within-tile, free = feature
        xsb = xpool.tile([128, KT * F_IN], F32)
        nc.sync.dma_start(
            xsb[:].rearrange("p (k f) -> p k f", k=KT),
            x_ap.rearrange("(k p) f -> p k f", p=128),
        )

        at_r = at_ap.rearrange("(k p) m -> p k m", p=128)
        psum_t = pst.tile([128, R], F32)
        for k in range(KT):
            at_k = apool.tile([128, R], F32, tag="atk")
            nc.sync.dma_start(at_k[:], at_r[:, k, :])
            nc.tensor.matmul(
                psum_t[:], xsb[:, k * F_IN:(k + 1) * F_IN], at_k[:],
                start=(k == 0), stop=(k == KT - 1),
            )
        tts = spool.tile([128, R], F32)
        nc.vector.tensor_copy(tts[:], psum_t[:])
        psum_e = pse.tile([F_EMB, R], F32)
        nc.tensor.matmul(psum_e[:], wsb[:], tts[:], start=True, stop=True)
        esb = spool.tile([F_EMB, R], F32)
        nc.scalar.activation(esb[:], psum_e[:],
                             mybir.ActivationFunctionType.Relu, bias=bsb[:])
        nc.sync.dma_start(et_ap[:], esb[:])

    nc.compile()
    return nc


def _build_phase2():
    """w_blk = sigmoid(lhsT_aug^T @ rhs_aug + bias_i) for this core's rows."""
    nc = bacc.Bacc("TRN2", target_bir_lowering=False, debug=False,
                   num_devices=NCORES)
    K = F_EMB + 1
    lh_ap = nc.dram_tensor("lh", [K, R], F32, kind="ExternalInput").ap()
    rh_ap = nc.dram_tensor("rh", [K, N], F32, kind="ExternalInput").ap()
    bi_ap = nc.dram_tensor("bi", [128, IT], F32, kind="ExternalInput").ap()
    wo_ap = nc.dram_tensor("wo", [R, N], F32, kind="ExternalOutput").ap()

    with tile.TileContext(nc) as tc, ExitStack() as ctx:
        inp = ctx.enter_context(tc.tile_pool(name="inp", bufs=1))
        psp = ctx.enter_context(tc.tile_pool(name="psp", bufs=4, space="PSUM"))
        outp = ctx.enter_context(tc.tile_pool(name="outp", bufs=4))

        lh = inp.tile([K, R], F32)
        nc.sync.dma_start(lh[:], lh_ap[:])
        rh = inp.tile([K, N], F32)
        nc.sync.dma_start(rh[:], rh_ap[:])
        bi = inp.tile([128, IT], F32)
        nc.sync.dma_start(bi[:], bi_ap[:])

        for i in range(IT):
            for j in range(JT):
                ps = psp.tile([128, 512], F32, tag="ps")
                nc.tensor.matmul(
                    ps[:], lh[:, i * 128:(i + 1) * 128],
                    rh[:, j * 512:(j + 1) * 512], start=True, stop=True,
                )
                ws = outp.tile([128, 512], F32, tag="ws")
                nc.scalar.activation(ws[:], ps[:],
                                     mybir.ActivationFunctionType.Sigmoid,
                                     bias=bi[:, i:i + 1])
                nc.sync.dma_start(
                    wo_ap[i * 128:(i + 1) * 128, j * 512:(j + 1) * 512], ws[:])

    nc.compile()
    return nc


def _get_nc(key, builder):
    nc = _NC_CACHE.get(key)
    if nc is None:
        nc = builder()
        _NC_CACHE[key] = nc
    return nc


def _edge_index() -> np.ndarray:
    idx = np.arange(B * N * N, dtype=np.int32)
    rows = idx // N
    cols = idx % N + N * (rows // N)
    return np.stack([rows, cols]).astype(np.int32)


def kernel(x, A, W_embed, b_embed, temperature, threshold):
    x = np.ascontiguousarray(np.asarray(x, dtype=np.float32))
    A = np.ascontiguousarray(np.asarray(A, dtype=np.float32))
    W_embed = np.ascontiguousarray(np.asarray(W_embed, dtype=np.float32))
    b_embed = np.ascontiguousarray(np.asarray(b_embed, dtype=np.float32))
    T = np.float32(np.asarray(temperature).reshape(()))
    thr = np.abs(np.float32(np.asarray(threshold).reshape(())))

    # ---- launch 1: x_emb ----
    nc1 = _get_nc("p1", _build_phase1)
    bb = b_embed.reshape(F_EMB, 1)
    in1 = []
    for c in range(NCORES):
        b, rb = divmod(c, CPB)
        at = np.ascontiguousarray(A[b, rb * R:(rb + 1) * R, :].T)
        in1.append({"at": at, "x": x[b], "w": W_embed, "bb": bb})
    res1 = run_bass_kernel_spmd(nc1, in1, core_ids=CORE_IDS)

    x_emb = np.empty((B, N, F_EMB), dtype=np.float32)
    for c in range(NCORES):
        b, rb = divmod(c, CPB)
        x_emb[b, rb * R:(rb + 1) * R, :] = res1.results[c]["et"].T

    # ---- host: global scale + fold constants ----
    centroid = x_emb.mean(axis=1, keepdims=True, dtype=np.float32)
    scale = np.float32(0.9) / np.abs(x_emb - centroid).max()
    s2 = np.float32(T * scale * scale)          # T * scale^2
    sq0 = np.einsum("bne,bne->bn", x_emb, x_emb).astype(np.float32)  # [B,N]

    nc2 = _get_nc("p2", _build_phase2)
    in2 = []
    for c in range(NCORES):
        b, rb = divmod(c, CPB)
        eT = x_emb[b].T                          # [E, N]
        lh = np.empty((F_EMB + 1, R), dtype=np.float32)
        lh[:F_EMB] = (2.0 * s2) * eT[:, rb * R:(rb + 1) * R]
        lh[F_EMB] = 1.0
        rh = np.empty((F_EMB + 1, N), dtype=np.float32)
        rh[:F_EMB] = eT
        rh[F_EMB] = (-s2) * sq0[b]
        bi = (T * thr - s2 * sq0[b, rb * R:(rb + 1) * R])
        bi = np.ascontiguousarray(bi.reshape(IT, 128).T)   # [128, IT]
        in2.append({"lh": lh, "rh": rh, "bi": bi})
    res2 = run_bass_kernel_spmd(nc2, in2, core_ids=CORE_IDS)

    adj = np.empty((B, N, N), dtype=np.float32)
    for c in range(NCORES):
        b, rb = divmod(c, CPB)
        adj[b, rb * R:(rb + 1) * R, :] = res2.results[c]["wo"]

    return x_emb, _edge_index(), adj.reshape(-1)
